# revision 5
# baseline (speedup 1.0000x reference)
"""DPLSTMLayer Trainium2 kernel: data-parallel over batch across 8 NeuronCores.

Layout notes (per core, batch slice of 8 samples):
  - gates computed as [g_partition, b_free]: stationary = w^T tiles [128 d|h, 128 g]
    (bf16 -> fast weight load), moving = x^T / h^T tiles [128, N].
  - hidden state h kept as hT [128 p, 8 k, 8 b] (p = hdim within tile, k = hdim tile)
    so the recurrence needs no transposes anywhere.
  - input projection (big GEMM over all T) precomputed into DRAM scratch gx
    with layout [T, 128 p, 4 G, 8 gt, 8 b], then streamed per step.
Outputs are produced in device layout and untransposed on the host.
"""
import sys

for _p in ("/opt/trn_rl_repo", "/root/.axon_site/_ro/trn_rl_repo"):
    if _p not in sys.path:
        sys.path.insert(0, _p)

import numpy as np
import ml_dtypes

T, B, D, H = 512, 64, 1024, 1024
NCORES = 8
BL = B // NCORES          # batch per core = 8
KD = D // 128             # 8 k-tiles over input dim
KH = H // 128             # 8 k-tiles over hidden dim
NG = 32                   # 4096 gates / 128
BF16 = ml_dtypes.bfloat16

_BUILD_CACHE = {}


def _build(t_steps):
    import concourse.bass as bass
    import concourse.tile as tile
    from concourse import mybir, bacc

    f32 = mybir.dt.float32
    bf16 = mybir.dt.bfloat16
    AF = mybir.ActivationFunctionType
    TB = t_steps * BL
    FT = min(512, TB)             # moving free-dim tile for the projection GEMM
    NM = TB // FT                 # number of f-tiles in projection
    TBLK = FT // BL               # timesteps covered per f-tile

    nc = bacc.Bacc("TRN2", target_bir_lowering=False, debug=False,
                   num_devices=NCORES)

    xT = nc.dram_tensor("xT", [128, KD, NM, FT], bf16, kind="ExternalInput")
    w_ih_in = nc.dram_tensor("w_ih_t", [128, KD, NG, 128], bf16, kind="ExternalInput")
    w_hh_in = nc.dram_tensor("w_hh_t", [128, KH, NG, 128], bf16, kind="ExternalInput")
    bias_in = nc.dram_tensor("bias_t", [128, NG], f32, kind="ExternalInput")
    h0_in = nc.dram_tensor("h0T", [128, KH, BL], bf16, kind="ExternalInput")
    c0_in = nc.dram_tensor("c0T", [128, KH, BL], f32, kind="ExternalInput")

    hs_out = nc.dram_tensor("hs", [t_steps, 128, KH, BL], f32, kind="ExternalOutput")
    cn_out = nc.dram_tensor("cn", [128, KH, BL], f32, kind="ExternalOutput")

    with tile.TileContext(nc) as tc:
        with tc.tile_pool(name="wpool", bufs=1) as wpool, \
             tc.tile_pool(name="state", bufs=1) as state, \
             tc.tile_pool(name="xin", bufs=2) as xin, \
             tc.tile_pool(name="aout", bufs=3) as aout, \
             tc.tile_pool(name="step", bufs=3) as step, \
             tc.tile_pool(name="gxp", bufs=4) as gxp, \
             tc.tile_pool(name="dram", bufs=1, space="DRAM") as dram:

            w_ih = wpool.tile([128, KD, NG, 128], bf16)
            nc.sync.dma_start(out=w_ih[:], in_=w_ih_in[:])
            w_hh = wpool.tile([128, KH, NG, 128], bf16)
            nc.sync.dma_start(out=w_hh[:], in_=w_hh_in[:])
            bias = wpool.tile([128, NG], f32)
            nc.sync.dma_start(out=bias[:], in_=bias_in[:])

            hT = state.tile([128, KH, BL], bf16, tag="hT")
            nc.sync.dma_start(out=hT[:], in_=h0_in[:])
            c_sb = state.tile([128, KH, BL], f32, tag="c")
            nc.sync.dma_start(out=c_sb[:], in_=c0_in[:])

            # one DRAM scratch block per f-tile so the recurrence for block m
            # only depends on projection block m
            gx_blocks = []
            for m in range(NM):
                gxb = dram.tile([TBLK, 128, 4, KH, BL], bf16, tag=f"gx{m}")
                gx_blocks.append(gxb)

            # ---------- phase A: gates_x = w_ih @ x^T (+bias), all timesteps ----
            with tc.tile_pool(name="psA", bufs=2, space="PSUM") as psA:
                for m in range(NM):
                    xts = []
                    for k in range(KD):
                        xt = xin.tile([128, FT], bf16, tag=f"xt{k}")
                        nc.sync.dma_start(out=xt[:], in_=xT[:, k, m, :])
                        xts.append(xt)
                    for gi in range(NG):
                        ps = psA.tile([128, FT], f32, tag=f"psA{gi % 2}")
                        for k in range(KD):
                            nc.tensor.matmul(ps[:], w_ih[:, k, gi, :], xts[k][:],
                                             start=(k == 0), stop=(k == KD - 1))
                        ob = aout.tile([128, TBLK, BL], bf16, tag="ob")
                        nc.scalar.activation(
                            out=ob.rearrange("p t b -> p (t b)"), in_=ps[:],
                            func=AF.Identity, bias=bias[:, gi:gi + 1], scale=1.0)
                        G, gt = gi // KH, gi % KH
                        nc.sync.dma_start(
                            out=gx_blocks[m][:, :, G, gt, :].rearrange(
                                "t p b -> p t b"),
                            in_=ob[:])

            # ---------- phase B: recurrence --------------------------------------
            with tc.tile_pool(name="psB", bufs=2, space="PSUM") as psB:
                for t in range(t_steps):
                    m, tt = t // TBLK, t % TBLK
                    gxt = gxp.tile([128, 4, KH, BL], bf16, tag="gxt")
                    nc.sync.dma_start(out=gxt[:], in_=gx_blocks[m][tt])
                    gxf = gxp.tile([128, 4, KH, BL], f32, tag="gxf")
                    nc.vector.tensor_copy(out=gxf[:], in_=gxt[:])

                    acts = []
                    for G in range(4):
                        ps = psB.tile([128, KH, BL], f32, tag=f"psB{G}")
                        for gt in range(KH):
                            for k in range(KH):
                                nc.tensor.matmul(
                                    ps[:, gt, :], w_hh[:, k, G * KH + gt, :],
                                    hT[:, k, :],
                                    start=(k == 0), stop=(k == KH - 1))
                        pre = step.tile([128, KH, BL], f32, tag=f"pre{G}")
                        nc.vector.tensor_add(out=pre[:], in0=ps[:], in1=gxf[:, G])
                        act = step.tile([128, KH, BL], f32, tag=f"act{G}")
                        nc.scalar.activation(
                            out=act[:], in_=pre[:],
                            func=(AF.Tanh if G == 2 else AF.Sigmoid))
                        acts.append(act)

                    a_i, a_f, a_g, a_o = acts
                    t_fc = step.tile([128, KH, BL], f32, tag="t_fc")
                    nc.vector.tensor_mul(out=t_fc[:], in0=a_f[:], in1=c_sb[:])
                    t_ig = step.tile([128, KH, BL], f32, tag="t_ig")
                    nc.vector.tensor_mul(out=t_ig[:], in0=a_i[:], in1=a_g[:])
                    nc.vector.tensor_add(out=c_sb[:], in0=t_fc[:], in1=t_ig[:])
                    th = step.tile([128, KH, BL], f32, tag="th")
                    nc.scalar.activation(out=th[:], in_=c_sb[:], func=AF.Tanh)
                    h_f = step.tile([128, KH, BL], f32, tag="h_f")
                    nc.vector.tensor_mul(out=h_f[:], in0=a_o[:], in1=th[:])
                    hT = state.tile([128, KH, BL], bf16, tag="hT")
                    nc.scalar.activation(out=hT[:], in_=h_f[:], func=AF.Identity)
                    nc.sync.dma_start(out=hs_out[t], in_=h_f[:])
                    if t == t_steps - 1:
                        nc.sync.dma_start(out=cn_out[:], in_=c_sb[:])
    nc.finalize()
    return nc


def _get_nc(t_steps):
    if t_steps not in _BUILD_CACHE:
        _BUILD_CACHE[t_steps] = _build(t_steps)
    return _BUILD_CACHE[t_steps]


def kernel(x, h0, c0, w_ih, b_ih, w_hh, b_hh, t_steps=T):
    from concourse.bass_utils import run_bass_kernel_spmd

    x = np.asarray(x, dtype=np.float32)
    h0 = np.asarray(h0, dtype=np.float32)
    c0 = np.asarray(c0, dtype=np.float32)
    w_ih = np.asarray(w_ih, dtype=np.float32)
    b_ih = np.asarray(b_ih, dtype=np.float32)
    w_hh = np.asarray(w_hh, dtype=np.float32)
    b_hh = np.asarray(b_hh, dtype=np.float32)

    ts = t_steps
    FTv = min(512, ts * BL)
    NMv = (ts * BL) // FTv
    TBLKv = FTv // BL

    # weights: [4H, D] -> transpose -> [128 p_d, KD, NG, 128 g] tiles, bf16
    def wprep(w, kdim):
        wt = np.ascontiguousarray(w.T).astype(BF16)          # [kdim*128, 4096]
        return np.ascontiguousarray(
            wt.reshape(kdim, 128, NG, 128).transpose(1, 0, 2, 3))

    w_ih_t = wprep(w_ih, KD)
    w_hh_t = wprep(w_hh, KH)
    bias_t = np.ascontiguousarray((b_ih + b_hh).reshape(NG, 128).T,
                                  dtype=np.float32)          # [128, NG]

    nc = _get_nc(ts)
    in_maps = []
    for j in range(NCORES):
        bs = slice(j * BL, (j + 1) * BL)
        xj = x[:ts, bs, :]                                   # [ts, BL, D]
        # -> [128 p, KD, NM, FT] where FT runs over (t_within_block, b)
        xTj = np.ascontiguousarray(
            xj.astype(BF16).transpose(2, 0, 1)               # [D, ts, BL]
            .reshape(KD, 128, NMv, TBLKv, BL)
            .transpose(1, 0, 2, 3, 4)
            .reshape(128, KD, NMv, FTv))
        h0j = np.ascontiguousarray(
            h0[bs].T.reshape(KH, 128, BL).transpose(1, 0, 2)).astype(BF16)
        c0j = np.ascontiguousarray(
            c0[bs].T.reshape(KH, 128, BL).transpose(1, 0, 2)).astype(np.float32)
        in_maps.append({
            "xT": xTj, "w_ih_t": w_ih_t, "w_hh_t": w_hh_t, "bias_t": bias_t,
            "h0T": h0j, "c0T": c0j,
        })

    res = run_bass_kernel_spmd(nc, in_maps, list(range(NCORES)))

    hs = np.empty((ts, B, H), dtype=np.float32)
    h_n = np.empty((B, H), dtype=np.float32)
    c_n = np.empty((B, H), dtype=np.float32)
    for j in range(NCORES):
        bs = slice(j * BL, (j + 1) * BL)
        hj = res.results[j]["hs"]                            # [ts, 128, KH, BL]
        hs[:, bs, :] = hj.transpose(0, 3, 2, 1).reshape(ts, BL, H)
        cj = res.results[j]["cn"]                            # [128, KH, BL]
        c_n[bs] = cj.transpose(2, 1, 0).reshape(BL, H)
    h_n[:] = hs[-1]
    return hs, h_n, c_n


# revision 12
# speedup vs baseline: 1.4459x; 1.4459x over previous
"""DPLSTMLayer Trainium2 kernel: data-parallel over batch across 8 NeuronCores.

Layout notes (per core, batch slice of 8 samples):
  - gates computed as [g_partition, b_free]: stationary = w^T tiles [128 d|h, 128 g]
    (bf16 -> fast weight load), moving = x^T / h^T tiles [128, N].
  - hidden state h kept as hT [128 p, 8 k, 8 b] (p = hdim within tile, k = hdim tile)
    so the recurrence needs no transposes anywhere.
  - input projection (big GEMM over all T) precomputed into DRAM scratch gx
    with layout [T, 128 p, 4 G, 8 gt, 8 b], then streamed per step.
Outputs are produced in device layout and untransposed on the host.
"""
import sys

for _p in ("/opt/trn_rl_repo", "/root/.axon_site/_ro/trn_rl_repo"):
    if _p not in sys.path:
        sys.path.insert(0, _p)

import numpy as np
import ml_dtypes

T, B, D, H = 512, 64, 1024, 1024
NCORES = 8
BL = B // NCORES          # batch per core = 8
KD = D // 128             # 8 k-tiles over input dim
KH = H // 128             # 8 k-tiles over hidden dim
NG = 32                   # 4096 gates / 128
BF16 = ml_dtypes.bfloat16

_BUILD_CACHE = {}


def _build(t_steps):
    import concourse.bass as bass
    import concourse.tile as tile
    from concourse import mybir, bacc

    f32 = mybir.dt.float32
    bf16 = mybir.dt.bfloat16
    AF = mybir.ActivationFunctionType
    TB = t_steps * BL
    FT = min(512, TB)             # moving free-dim tile for the projection GEMM
    NM = TB // FT                 # number of f-tiles in projection
    TBLK = FT // BL               # timesteps covered per f-tile

    nc = bacc.Bacc("TRN2", target_bir_lowering=False, debug=False,
                   num_devices=NCORES)

    xT = nc.dram_tensor("xT", [128, KD, NM, FT], bf16, kind="ExternalInput")
    w_ih_in = nc.dram_tensor("w_ih_t", [128, KD, NG, 128], bf16, kind="ExternalInput")
    w_hh_in = nc.dram_tensor("w_hh_t", [128, KH, NG, 128], bf16, kind="ExternalInput")
    bias_in = nc.dram_tensor("bias_t", [128, NG], f32, kind="ExternalInput")
    h0_in = nc.dram_tensor("h0T", [128, KH, BL], bf16, kind="ExternalInput")
    c0_in = nc.dram_tensor("c0T", [128, KH, BL], f32, kind="ExternalInput")

    hs_out = nc.dram_tensor("hs", [t_steps, 128, KH, BL], f32, kind="ExternalOutput")
    cn_out = nc.dram_tensor("cn", [128, KH, BL], f32, kind="ExternalOutput")

    with tile.TileContext(nc) as tc:
        with tc.tile_pool(name="wpool", bufs=1) as wpool, \
             tc.tile_pool(name="state", bufs=1) as state, \
             tc.tile_pool(name="xin", bufs=2) as xin, \
             tc.tile_pool(name="aout", bufs=3) as aout, \
             tc.tile_pool(name="step", bufs=3) as step, \
             tc.tile_pool(name="gxp", bufs=4) as gxp, \
             tc.tile_pool(name="dram", bufs=1, space="DRAM") as dram:

            w_ih = wpool.tile([128, KD, NG, 128], bf16)
            nc.sync.dma_start(out=w_ih[:], in_=w_ih_in[:])
            w_hh = wpool.tile([128, KH, NG, 128], bf16)
            nc.sync.dma_start(out=w_hh[:], in_=w_hh_in[:])
            bias = wpool.tile([128, NG], f32)
            nc.sync.dma_start(out=bias[:], in_=bias_in[:])

            hT = state.tile([128, KH, BL], bf16, tag="hT")
            nc.sync.dma_start(out=hT[:], in_=h0_in[:])
            c_sb = state.tile([128, KH, BL], f32, tag="c")
            nc.sync.dma_start(out=c_sb[:], in_=c0_in[:])

            # one DRAM scratch block per f-tile so the recurrence for block m
            # only depends on projection block m
            gx_blocks = []
            for m in range(NM):
                gxb = dram.tile([TBLK, 128, 4, KH, BL], bf16, tag=f"gx{m}")
                gx_blocks.append(gxb)

            # ---------- phase A: gates_x = w_ih @ x^T (+bias), all timesteps ----
            with tc.tile_pool(name="psA", bufs=2, space="PSUM") as psA:
                for m in range(NM):
                    xts = []
                    for k in range(KD):
                        xt = xin.tile([128, FT], bf16, tag=f"xt{k}")
                        nc.sync.dma_start(out=xt[:], in_=xT[:, k, m, :])
                        xts.append(xt)
                    for gi in range(NG):
                        ps = psA.tile([128, FT], f32, tag=f"psA{gi % 2}")
                        for k in range(KD):
                            nc.tensor.matmul(ps[:], w_ih[:, k, gi, :], xts[k][:],
                                             start=(k == 0), stop=(k == KD - 1))
                        ob = aout.tile([128, TBLK, BL], bf16, tag="ob")
                        nc.scalar.activation(
                            out=ob.rearrange("p t b -> p (t b)"), in_=ps[:],
                            func=AF.Identity, bias=bias[:, gi:gi + 1], scale=1.0)
                        G, gt = gi // KH, gi % KH
                        nc.sync.dma_start(
                            out=gx_blocks[m][:, :, G, gt, :].rearrange(
                                "t p b -> p t b"),
                            in_=ob[:])

            # ---------- phase B: recurrence --------------------------------------
            with tc.tile_pool(name="psB", bufs=2, space="PSUM") as psB:
                for t in range(t_steps):
                    m, tt = t // TBLK, t % TBLK
                    gxt = gxp.tile([128, 4, KH, BL], bf16, tag="gxt")
                    nc.sync.dma_start(out=gxt[:], in_=gx_blocks[m][tt])
                    gxf = gxp.tile([128, 4, KH, BL], f32, tag="gxf")
                    nc.vector.tensor_copy(out=gxf[:], in_=gxt[:])

                    acts = []
                    for G in range(4):
                        ps = psB.tile([128, KH, BL], f32, tag=f"psB{G}")
                        for gt in range(KH):
                            for k in range(KH):
                                nc.tensor.matmul(
                                    ps[:, gt, :], w_hh[:, k, G * KH + gt, :],
                                    hT[:, k, :],
                                    start=(k == 0), stop=(k == KH - 1))
                        pre = step.tile([128, KH, BL], f32, tag=f"pre{G}")
                        nc.vector.tensor_add(out=pre[:], in0=ps[:], in1=gxf[:, G])
                        act = step.tile([128, KH, BL], f32, tag=f"act{G}")
                        nc.scalar.activation(
                            out=act[:], in_=pre[:],
                            func=(AF.Tanh if G == 2 else AF.Sigmoid))
                        acts.append(act)

                    a_i, a_f, a_g, a_o = acts
                    t_fc = step.tile([128, KH, BL], f32, tag="t_fc")
                    nc.vector.tensor_mul(out=t_fc[:], in0=a_f[:], in1=c_sb[:])
                    t_ig = step.tile([128, KH, BL], f32, tag="t_ig")
                    nc.vector.tensor_mul(out=t_ig[:], in0=a_i[:], in1=a_g[:])
                    nc.vector.tensor_add(out=c_sb[:], in0=t_fc[:], in1=t_ig[:])
                    th = step.tile([128, KH, BL], f32, tag="th")
                    nc.scalar.activation(out=th[:], in_=c_sb[:], func=AF.Tanh)
                    h_f = step.tile([128, KH, BL], f32, tag="h_f")
                    nc.vector.tensor_mul(out=h_f[:], in0=a_o[:], in1=th[:])
                    hT = state.tile([128, KH, BL], bf16, tag="hT")
                    nc.scalar.activation(out=hT[:], in_=h_f[:], func=AF.Identity)
                    nc.sync.dma_start(out=hs_out[t], in_=h_f[:])
                    if t == t_steps - 1:
                        nc.sync.dma_start(out=cn_out[:], in_=c_sb[:])
    nc.finalize()
    return nc


def _get_runner(t_steps):
    """Build the Bass program once and wrap it in a cached jitted shard_map
    callable so repeat calls skip re-tracing/recompiling."""
    if t_steps in _BUILD_CACHE:
        return _BUILD_CACHE[t_steps]

    import jax
    import numpy as np_
    from jax.sharding import Mesh, PartitionSpec
    from jax.experimental.shard_map import shard_map
    from concourse import bass2jax, mybir
    import concourse.bass as bass

    nc = _build(t_steps)
    bass2jax.install_neuronx_cc_hook()

    part_name = nc.partition_id_tensor.name if nc.partition_id_tensor else None
    in_names, out_names, out_avals, zero_shapes = [], [], [], []
    for alloc in nc.m.functions[0].allocations:
        if not isinstance(alloc, mybir.MemoryLocationSet):
            continue
        name = alloc.memorylocations[0].name
        if alloc.kind == "ExternalInput":
            if name != part_name:
                in_names.append(name)
        elif alloc.kind == "ExternalOutput":
            out_names.append(name)
            shape = tuple(alloc.tensor_shape)
            dt_np = mybir.dt.np(alloc.dtype)
            out_avals.append(jax.core.ShapedArray(shape, dt_np))
            zero_shapes.append((shape, dt_np))
    n_params = len(in_names)
    all_names = in_names + out_names
    if part_name is not None:
        all_names = all_names + [part_name]

    def _body(*args):
        operands = list(args)
        if part_name is not None:
            operands.append(bass2jax.partition_id_tensor())
        outs = bass2jax._bass_exec_p.bind(
            *operands,
            out_avals=tuple(out_avals),
            in_names=tuple(all_names),
            out_names=tuple(out_names),
            lowering_input_output_aliases=(),
            sim_require_finite=True,
            sim_require_nnan=True,
            nc=nc,
        )
        return tuple(outs)

    devices = jax.devices()[:NCORES]
    mesh = Mesh(np_.asarray(devices), ("core",))
    n_outs = len(out_names)
    sharded = jax.jit(
        shard_map(_body, mesh=mesh,
                  in_specs=(PartitionSpec("core"),) * (n_params + n_outs),
                  out_specs=(PartitionSpec("core"),) * n_outs,
                  check_rep=False),
        donate_argnums=tuple(range(n_params, n_params + n_outs)),
        keep_unused=True,
    )

    def run(in_maps):
        concat_in = [
            np_.concatenate([np_.asarray(m[name]) for m in in_maps], axis=0)
            for name in in_names
        ]
        concat_zeros = [
            np_.zeros((NCORES * s[0], *s[1:]), d) for s, d in zero_shapes
        ]
        out_arrs = sharded(*concat_in, *concat_zeros)
        return [
            {name: np_.asarray(out_arrs[i]).reshape(NCORES, *out_avals[i].shape)[c]
             for i, name in enumerate(out_names)}
            for c in range(NCORES)
        ]

    _BUILD_CACHE[t_steps] = run
    return run


def kernel(x, h0, c0, w_ih, b_ih, w_hh, b_hh, t_steps=T):
    x = np.asarray(x, dtype=np.float32)
    h0 = np.asarray(h0, dtype=np.float32)
    c0 = np.asarray(c0, dtype=np.float32)
    w_ih = np.asarray(w_ih, dtype=np.float32)
    b_ih = np.asarray(b_ih, dtype=np.float32)
    w_hh = np.asarray(w_hh, dtype=np.float32)
    b_hh = np.asarray(b_hh, dtype=np.float32)

    ts = t_steps
    FTv = min(512, ts * BL)
    NMv = (ts * BL) // FTv
    TBLKv = FTv // BL

    # weights: [4H, D] -> transpose -> [128 p_d, KD, NG, 128 g] tiles, bf16
    def wprep(w, kdim):
        wt = np.ascontiguousarray(w.T).astype(BF16)          # [kdim*128, 4096]
        return np.ascontiguousarray(
            wt.reshape(kdim, 128, NG, 128).transpose(1, 0, 2, 3))

    w_ih_t = wprep(w_ih, KD)
    w_hh_t = wprep(w_hh, KH)
    bias_t = np.ascontiguousarray((b_ih + b_hh).reshape(NG, 128).T,
                                  dtype=np.float32)          # [128, NG]

    run = _get_runner(ts)
    in_maps = []
    for j in range(NCORES):
        bs = slice(j * BL, (j + 1) * BL)
        xj = x[:ts, bs, :]                                   # [ts, BL, D]
        # -> [128 p, KD, NM, FT] where FT runs over (t_within_block, b)
        xTj = np.ascontiguousarray(
            xj.astype(BF16).transpose(2, 0, 1)               # [D, ts, BL]
            .reshape(KD, 128, NMv, TBLKv, BL)
            .transpose(1, 0, 2, 3, 4)
            .reshape(128, KD, NMv, FTv))
        h0j = np.ascontiguousarray(
            h0[bs].T.reshape(KH, 128, BL).transpose(1, 0, 2)).astype(BF16)
        c0j = np.ascontiguousarray(
            c0[bs].T.reshape(KH, 128, BL).transpose(1, 0, 2)).astype(np.float32)
        in_maps.append({
            "xT": xTj, "w_ih_t": w_ih_t, "w_hh_t": w_hh_t, "bias_t": bias_t,
            "h0T": h0j, "c0T": c0j,
        })

    results = run(in_maps)

    hs = np.empty((ts, B, H), dtype=np.float32)
    h_n = np.empty((B, H), dtype=np.float32)
    c_n = np.empty((B, H), dtype=np.float32)
    for j in range(NCORES):
        bs = slice(j * BL, (j + 1) * BL)
        hj = results[j]["hs"]                                # [ts, 128, KH, BL]
        hs[:, bs, :] = hj.transpose(0, 3, 2, 1).reshape(ts, BL, H)
        cj = results[j]["cn"]                                # [128, KH, BL]
        c_n[bs] = cj.transpose(2, 1, 0).reshape(BL, H)
    h_n[:] = hs[-1]
    return hs, h_n, c_n
